# revision 1
# baseline (speedup 1.0000x reference)
"""Trainium2 Bass kernel for nn_FFMLP (4-layer MLP, hidden=128, relu).

V2 strategy (pure data parallel, batch sharded 8 ways):
- Feature-major on-chip layout: activations live as [feat, batch]; every layer
  is one K<=128 fp16 matmul per 512-col chunk (weights stationary, activation
  stream moving; fp32 PSUM).
- Quad-strip input layout [128, B/4] (4 row-tiled K=32 strips) so input DMA
  uses all 128 partitions.
- Layer-phased groups of 8 chunks: the PE runs one layer at a time within a
  group, so stationary weights reload only at phase switches (~12 LdWeights
  per group after band-aware dedup, vs ~4 per chunk fine-grained).
- PSUM is one 8-bank rotation of four [128,1024] (2-bank) block tiles. Each
  block = 2 chunks of one layer; its relu+downcast evacuation is ONE FD=1024
  instruction assigned greedily to ScalarE or VectorE (both read PSUM at
  1 elem/cycle/lane on TRN2; ScalarE 1.2GHz vs VectorE 0.96GHz, so the greedy
  split lands ~53/47) -- the two evac engines are the roofline here.
- L4 (M=16) packs 8 chunks into one block via column tiling (tile_position
  (0,32j)); its matmul pairs are interleaved between the NEXT group's L0
  blocks and its evacuation is split into two FD=512 copies emitted
  mid-phase, so evac demand stays uniform (a solid 8-MM L4 burst starves the
  evac engines ~1.7us). Output is DMA'd quad-packed fp16; the host unpacks.
- One L0 block of the next group is computed at the end of the evac-bound P2
  phase ("carry"), leveling PE load across phases (P0 is otherwise PE-bound).
- The final group's L4 uses two rotation tiles so its two output copies read
  independent sources (cross-engine readers of one tile serialize).
- Output yt is fp16 (host casts to fp32): halves output DMA.
"""
import sys

if "/opt/trn_rl_repo" not in sys.path:
    sys.path.insert(0, "/opt/trn_rl_repo")

import numpy as np

import concourse.bass as bass
import concourse.mybir as mybir
import concourse.tile as tile

INPUT_DIM = 32
OUTPUT_DIM = 16
HIDDEN = 128
PADDED_OUT = 16
NUM_LAYERS = 4
B = 524288
N_CORES = 8
B_CORE = B // N_CORES  # 65536
CHUNK = 512
N_CHUNKS = B_CORE // CHUNK  # 128
GROUP = 8  # chunks per layer-phase group
N_WARM = 20  # PE p-state warm-up matmuls (FD=128)

fp16 = mybir.dt.float16
fp32 = mybir.dt.float32
RELU = mybir.ActivationFunctionType.Relu

# cost-model ns for the greedy evac balancer (TRN2: ACT 1.2GHz +222cyc init,
# DVE 0.96GHz +120cyc init; both 1 elem/cycle/lane from fp32 PSUM)
ACT_EVAC_NS = (1024 + 222) / 1.2
DVE_EVAC_NS = (1024 + 120) / 0.96


def _split_waits(nc, max_waits=1):
    """walrus in this image rejects >1 semaphore wait per instruction on some
    formats; split excess waits onto preceding NOPs on the same engine queue
    (queues are in-order, so semantics are preserved)."""
    n_new = 0
    for bb in nc.main_func.blocks:
        out_list = []
        changed = False
        for ins in bb.instructions:
            si = ins.sync_info
            if si is not None and si.on_wait and len(si.on_wait) > max_waits:
                waits = list(si.on_wait)
                extra, keep = waits[:-max_waits], waits[-max_waits:]
                while extra:
                    chunk, extra = extra[:max_waits], extra[max_waits:]
                    n_new += 1
                    nop = mybir.InstNoOp(name=f"I-waitsplit-{n_new}", ins=[], outs=[])
                    nop.engine = ins.engine
                    nop.sync_info = mybir.SyncInfo(on_wait=chunk, on_update=[])
                    out_list.append(nop)
                ins.sync_info = mybir.SyncInfo(on_wait=keep, on_update=si.on_update)
                changed = True
            out_list.append(ins)
        if changed:
            bb.instructions = out_list
    return n_new


def _rect_of(ins):
    """PE-array rectangle (r0, r1, c0, c1) occupied by an InstLdweights."""
    tp = ins.tile_position or (0, 0)
    ts = getattr(ins, "tile_size", None) or (128, 128)
    r0, c0 = int(tp[0]), int(tp[1])
    kr, mc = int(ts[0]), int(ts[1])
    return (r0, r0 + kr, c0, c0 + mc)


def _dedup_ldweights(nc):
    """Band-aware LdWeights dedup: the PE array retains weights per tile
    rectangle; a load whose (weights AP, position, size, mode) matches what is
    already resident in that rectangle -- and has not been overlapped by a
    later load -- is replaced with a NOP carrying the same sync_info."""
    n = 0
    for bb in nc.main_func.blocks:
        il = list(bb.instructions)
        resident = {}  # (r0, c0) -> (key, rect)
        changed = False
        for idx, ins in enumerate(il):
            if ins.engine != mybir.EngineType.PE:
                continue
            if isinstance(ins, mybir.InstLdweights):
                rect = _rect_of(ins)
                key = (
                    repr(ins.ins[0]),
                    str(ins.tile_position),
                    str(getattr(ins, "tile_size", None)),
                    str(ins.perf_mode),
                    bool(ins.is_transpose),
                )
                pos = (rect[0], rect[2])
                cur = resident.get(pos)
                if cur is not None and cur[0] == key:
                    nop = mybir.InstNoOp(name=ins.name, ins=[], outs=[])
                    nop.engine = ins.engine
                    nop.sync_info = ins.sync_info
                    il[idx] = nop
                    changed = True
                    n += 1
                    continue
                # evict any resident rectangle this load overlaps
                for p, (k, rc) in list(resident.items()):
                    if rect[0] < rc[1] and rc[0] < rect[1] and rect[2] < rc[3] and rc[2] < rect[3]:
                        del resident[p]
                resident[pos] = (key, rect)
        if changed:
            bb.instructions = il
    return n


def build(n_chunks=N_CHUNKS):
    nc = bass.Bass()
    ncols = n_chunks * CHUNK
    nquad = ncols // 4
    n_groups = n_chunks // GROUP
    assert n_chunks % GROUP == 0 and GROUP == 8

    # xt quad-strip: xt[32*j + f, q*512 + c] = x.T[f, (4q+j)*512 + c]
    xt = nc.declare_dram_parameter("xt", [128, nquad], fp16, isOutput=False)
    w0 = nc.declare_dram_parameter("w0", [128, HIDDEN], fp16, isOutput=False)
    w1 = nc.declare_dram_parameter("w1", [HIDDEN, HIDDEN], fp16, isOutput=False)
    w2 = nc.declare_dram_parameter("w2", [HIDDEN, HIDDEN], fp16, isOutput=False)
    w3 = nc.declare_dram_parameter("w3", [HIDDEN, HIDDEN], fp16, isOutput=False)
    w4 = nc.declare_dram_parameter("w4", [HIDDEN, PADDED_OUT], fp16, isOutput=False)
    # yt quad-packed: yt[32*j + r, q*512 + c] = out[(4q+j)*512 + c, r], r<16
    yt = nc.declare_dram_parameter("yt", [128, nquad], fp16, isOutput=True)

    with tile.TileContext(nc) as tc:
        with (
            tc.tile_pool(name="wp", bufs=1) as wp,
            tc.tile_pool(name="io", bufs=1) as io,
            tc.tile_pool(name="hp", bufs=1) as hp,
            tc.tile_pool(name="op", bufs=1) as op,
            tc.tile_pool(name="ps", bufs=1, space="PSUM") as ps,
        ):
            # HAM warm-up source: memset (no DMA dependency, PE can start
            # ramping immediately)
            wwarm = wp.tile([128, 128], fp16, tag="wm", name="wwarm")
            nc.vector.memset(wwarm[:, :], 0.0)

            w0s = wp.tile([128, HIDDEN], fp16, tag="w0", name="w0s")
            w1s = wp.tile([HIDDEN, HIDDEN], fp16, tag="w1", name="w1s")
            w2s = wp.tile([HIDDEN, HIDDEN], fp16, tag="w2", name="w2s")
            w3s = wp.tile([HIDDEN, HIDDEN], fp16, tag="w3", name="w3s")
            w4s = wp.tile([HIDDEN, PADDED_OUT], fp16, tag="w4", name="w4s")

            def blk_tile(name):
                return ps.tile([128, 1024], fp32, tag="blk", bufs=4, name=name)

            def l4_tile():
                return blk_tile("pl4")

            pwarm = blk_tile("pwarm")
            for _ in range(N_WARM):
                nc.tensor.matmul(
                    pwarm[:, 0:128], wwarm[:, :], wwarm[:, 0:128],
                    start=True, stop=True,
                )

            # greedy two-engine evac balancer
            bal = {"act": 0.0, "dve": 0.0}

            def evac(dst, src, relu, fd=1024):
                act_ns = (fd + 222) / 1.2
                dve_ns = (fd + 120) / 0.96
                use_act = bal["act"] + act_ns <= bal["dve"] + dve_ns
                if use_act:
                    bal["act"] += act_ns
                    if relu:
                        nc.scalar.activation(dst, src, RELU)
                    else:
                        nc.scalar.copy(out=dst, in_=src)
                else:
                    bal["dve"] += dve_ns
                    if relu:
                        nc.vector.tensor_scalar_max(dst, src, 0.0)
                    else:
                        nc.vector.tensor_copy(dst, src)

            slabs = {}

            def load_slab(g):
                if g >= n_groups:
                    return
                W = GROUP * 128
                xs = io.tile([128, W], fp16, tag="xin", bufs=6, name="xs")
                nc.sync.dma_start(out=xs, in_=xt[:, g * W : (g + 1) * W])
                slabs[g] = xs

            def l4_mm(blk, pairs, i):
                """One L4 matmul: chunk i of its group into [128,1024] block
                rows 32j (j=i%4), col half qh=i//4."""
                j, qh = i % 4, i // 4
                src = pairs[i // 2]
                nc.tensor.matmul(
                    blk[32 * j : 32 * j + PADDED_OUT, qh * 512 : (qh + 1) * 512],
                    w4s[:, :],
                    src[:, (i % 2) * 512 : (i % 2 + 1) * 512],
                    start=True,
                    stop=True,
                    tile_position=(0, 32 * j),
                )

            # DMA order: first input slab first (the long pole for the first
            # real matmul), weights interleaved in first-use order.
            load_slab(0)
            nc.sync.dma_start(out=w0s, in_=w0[:, :])
            nc.sync.dma_start(out=w1s, in_=w1[:, :])
            load_slab(1)
            nc.sync.dma_start(out=w2s, in_=w2[:, :])
            nc.sync.dma_start(out=w3s, in_=w3[:, :])
            nc.sync.dma_start(out=w4s, in_=w4[:, :])
            h4_prev = None
            h1_carry = None

            for g in range(n_groups):
                load_slab(g + 2)
                xs = slabs.pop(g)
                h = {}  # (layer, pair) -> SBUF pair tile [128, 1024]

                # The previous group's 8 L4 matmuls are interleaved between
                # this group's L0 blocks with split mid-phase copies, keeping
                # evac production uniform (a solid 8-MM L4 block starves the
                # two evac engines ~1.7us).
                l4_blk = l4_tile() if h4_prev is not None else None

                # ---- P0: L0 (K=32 strips); block 0 may have been carried
                # into the previous group's P3 (levels PE load: P0 was
                # PE-bound, P3 evac-bound)
                def l0_block(m, xs_, into):
                    blk = blk_tile("p0")
                    for half in range(2):
                        i = 2 * m + half
                        j, ql = i % 4, i // 4
                        nc.tensor.matmul(
                            blk[:, half * 512 : (half + 1) * 512],
                            w0s[32 * j : 32 * j + INPUT_DIM, :],
                            xs_[32 * j : 32 * j + INPUT_DIM, ql * 512 : (ql + 1) * 512],
                            start=True,
                            stop=True,
                            tile_position=(32 * j, 0),
                        )
                    h1 = hp.tile([128, 1024], fp16, tag="h1", bufs=16, name="h1")
                    evac(h1[:, :], blk[:, :], relu=True)
                    into[(1, m)] = h1

                m0 = 0
                if h1_carry is not None:
                    h[(1, 0)] = h1_carry
                    h1_carry = None
                    m0 = 1
                n_l4 = 0
                l4_osb = None
                for m in range(m0, GROUP // 2):
                    l0_block(m, xs, h)
                    if l4_blk is not None:
                        take = min(2 if m >= GROUP // 2 - 2 else 1, 4 - n_l4)
                        for _ in range(take):
                            l4_mm(l4_blk, h4_prev, 2 * n_l4)
                            l4_mm(l4_blk, h4_prev, 2 * n_l4 + 1)
                            n_l4 += 1
                            if n_l4 == 2:
                                l4_osb = op.tile(
                                    [128, 1024], fp16, tag="osb", bufs=8, name="osb"
                                )
                                evac(l4_osb[:, 0:512], l4_blk[:, 0:512], relu=False, fd=512)
                if l4_blk is not None:
                    while n_l4 < 4:
                        l4_mm(l4_blk, h4_prev, 2 * n_l4)
                        l4_mm(l4_blk, h4_prev, 2 * n_l4 + 1)
                        n_l4 += 1
                    evac(l4_osb[:, 512:1024], l4_blk[:, 512:1024], relu=False, fd=512)
                    q0 = 2 * (g - 1)
                    nc.sync.dma_start(out=yt[:, q0 * 512 : (q0 + 2) * 512], in_=l4_osb[:, :])
                    l4_blk = None

                # ---- P1..P3: L1..L3 (K=128)
                for layer, ws in ((1, w1s), (2, w2s), (3, w3s)):
                    for m in range(GROUP // 2):
                        blk = blk_tile(f"p{layer}")
                        src = h[(layer, m)]
                        for half in range(2):
                            nc.tensor.matmul(
                                blk[:, half * 512 : (half + 1) * 512],
                                ws[:, :],
                                src[:, half * 512 : (half + 1) * 512],
                                start=True,
                                stop=True,
                            )
                        hn = hp.tile(
                            [128, 1024], fp16, tag=f"h{layer + 1}", bufs=16,
                            name=f"h{layer + 1}",
                        )
                        evac(hn[:, :], blk[:, :], relu=True)
                        h[(layer + 1, m)] = hn
                    if layer == 2 and g + 1 < n_groups:
                        # level PE load: compute the next group's first L0
                        # block in this evac-bound phase (P0 is PE-bound)
                        carry = {}
                        l0_block(0, slabs[g + 1], carry)
                        h1_carry = carry[(1, 0)]
                h4_prev = [h[(4, m)] for m in range(GROUP // 2)]

            # tail: two rotation tiles (4 chunks each, cols 0:512) so the two
            # copies have independent sources and run fully in parallel
            l4_a = l4_tile()
            q0 = 2 * (n_groups - 1)
            for i in range(4):
                l4_mm(l4_a, h4_prev, i)
            l4_b = l4_tile()
            for i in range(4, GROUP):
                l4_mm(l4_b, h4_prev[2:], i - 4)
            osbA = op.tile([128, 512], fp16, tag="osA", bufs=1, name="osbA")
            osbB = op.tile([128, 512], fp16, tag="osB", bufs=1, name="osbB")
            nc.vector.tensor_copy(osbA[:, :], l4_a[:, 0:512])
            nc.scalar.copy(out=osbB[:, :], in_=l4_b[:, 0:512])
            nc.sync.dma_start(out=yt[:, q0 * 512 : (q0 + 1) * 512], in_=osbA[:, :])
            nc.sync.dma_start(out=yt[:, (q0 + 1) * 512 : (q0 + 2) * 512], in_=osbB[:, :])
    _dedup_ldweights(nc)
    _split_waits(nc)
    return nc


def _split_weights(weights):
    ws = []
    off = 0
    ws.append(weights[off : off + HIDDEN * INPUT_DIM].reshape(HIDDEN, INPUT_DIM))
    off += HIDDEN * INPUT_DIM
    for _ in range(NUM_LAYERS - 1):
        ws.append(weights[off : off + HIDDEN * HIDDEN].reshape(HIDDEN, HIDDEN))
        off += HIDDEN * HIDDEN
    ws.append(weights[off : off + PADDED_OUT * HIDDEN].reshape(PADDED_OUT, HIDDEN))
    return ws


_NC_CACHE = {}


def make_in_maps(inputs: np.ndarray, weights: np.ndarray):
    ws = _split_weights(np.asarray(weights, dtype=np.float32))
    w0t = np.ascontiguousarray(ws[0].T).astype(np.float16)  # [32, 128]
    wmaps = {
        "w0": np.concatenate([w0t] * 4, axis=0),  # [128, 128], 4 strips
        "w1": np.ascontiguousarray(ws[1].T).astype(np.float16),
        "w2": np.ascontiguousarray(ws[2].T).astype(np.float16),
        "w3": np.ascontiguousarray(ws[3].T).astype(np.float16),
        "w4": np.ascontiguousarray(ws[4].T).astype(np.float16),
    }
    in_maps = []
    for i in range(N_CORES):
        xc = inputs[i * B_CORE : (i + 1) * B_CORE]
        xtc = np.ascontiguousarray(xc.T).astype(np.float16)  # [32, B_CORE]
        # quad-strip: [128, B_CORE/4]
        xq = np.ascontiguousarray(
            xtc.reshape(INPUT_DIM, N_CHUNKS // 4, 4, CHUNK)
            .transpose(2, 0, 1, 3)
            .reshape(128, B_CORE // 4)
        )
        in_maps.append({"xt": xq, **wmaps})
    return in_maps


def kernel(inputs: np.ndarray, weights: np.ndarray) -> np.ndarray:
    from concourse.bass_utils import run_bass_kernel_spmd

    assert inputs.shape == (B, INPUT_DIM), inputs.shape
    in_maps = make_in_maps(inputs, weights)
    if "nc" not in _NC_CACHE:
        _NC_CACHE["nc"] = build()
    nc = _NC_CACHE["nc"]
    res = run_bass_kernel_spmd(nc, in_maps, list(range(N_CORES)))
    outs = []
    for r in res.results:
        yq = r["yt"]  # [128, B_CORE/4] fp16 quad-packed
        o = (
            yq.reshape(4, 32, N_CHUNKS // 4, CHUNK)[:, :PADDED_OUT]
            .transpose(2, 0, 3, 1)
            .reshape(B_CORE, PADDED_OUT)
        )
        outs.append(o.astype(np.float32))
    return np.concatenate(outs, axis=0)[:, :OUTPUT_DIM]



# revision 3
# speedup vs baseline: 1.0041x; 1.0041x over previous
"""Trainium2 Bass kernel for nn_FFMLP (4-layer MLP, hidden=128, relu) — v3.

Pure data parallel over 8 cores (B/8 = 65536 samples each). Feature-major
activations [128, cols]; weights stationary for L0-L3 (one fp16 matmul per
512-col chunk, fp32 PSUM). Key structural points:

- L4 FLIPPED: stationary = h4 chunk [K=128, M=128 samples], moving = W4^T
  [128, 16] -> output [128 samples, 16] per chunk. Full-partition output
  means FD=16 per 128 samples: L4 costs 512 PSUM cols/group instead of
  4096 (the old quad-packed form wasted 8x on M=16 outputs), and its
  evacuation halves to 512 cols/group.
- Flat software-pipelined stream: per group, units of [128,1024] PSUM
  (2 FD512 matmuls + 1 FD1024 evac) flow through a 3-deep PSUM rotation;
  L4 PSUM is a single [128,1024] tile filled 512/group and evacuated
  (copy, no relu) once per 2 groups. 3*1024 + 1024 fp32 = 16KB = all 8
  PSUM banks.
- Evac (the roofline: every PSUM byte must exit via ACT or DVE; DMA and
  GPSIMD have no PSUM route): greedy two-engine balance, ACT (FD+222cyc)
  @1.2GHz vs DVE (FD+120cyc)@0.96GHz.
- Output: osb [128, 1024] fp16 per 2 groups -> one DMA each; host applies
  the inverse packing permutation.
"""
import sys

if "/opt/trn_rl_repo" not in sys.path:
    sys.path.insert(0, "/opt/trn_rl_repo")

import numpy as np

import concourse.bass as bass
import concourse.mybir as mybir
import concourse.tile as tile

INPUT_DIM = 32
OUTPUT_DIM = 16
HIDDEN = 128
NUM_LAYERS = 4
B = 524288
N_CORES = 8
B_CORE = B // N_CORES  # 65536
CHUNK = 512
N_CHUNKS = B_CORE // CHUNK  # 128
GROUP = 8  # 512-col chunks per group (4096 samples)
N_GROUPS = N_CHUNKS // GROUP  # 16
N_WARM = 26

fp16 = mybir.dt.float16
fp32 = mybir.dt.float32
RELU = mybir.ActivationFunctionType.Relu

# evac cost model (ns) for the greedy balancer
ACT_NS = lambda fd: (fd + 222) / 1.2
DVE_NS = lambda fd: (fd + 120) / 0.96


def _split_waits(nc, max_waits=1):
    """walrus in this image rejects >1 semaphore wait per instruction on some
    formats; split excess waits onto preceding NOPs on the same engine queue
    (queues are in-order, so semantics are preserved)."""
    n_new = 0
    for bb in nc.main_func.blocks:
        out_list = []
        changed = False
        for ins in bb.instructions:
            si = ins.sync_info
            if si is not None and si.on_wait and len(si.on_wait) > max_waits:
                waits = list(si.on_wait)
                extra, keep = waits[:-max_waits], waits[-max_waits:]
                while extra:
                    chunk, extra = extra[:max_waits], extra[max_waits:]
                    n_new += 1
                    nop = mybir.InstNoOp(name=f"I-waitsplit-{n_new}", ins=[], outs=[])
                    nop.engine = ins.engine
                    nop.sync_info = mybir.SyncInfo(on_wait=chunk, on_update=[])
                    out_list.append(nop)
                ins.sync_info = mybir.SyncInfo(on_wait=keep, on_update=si.on_update)
                changed = True
            out_list.append(ins)
        if changed:
            bb.instructions = out_list
    return n_new


def build(n_groups=N_GROUPS):
    nc = bass.Bass()
    ncols = n_groups * GROUP * CHUNK
    nquad = ncols // 4

    xt = nc.declare_dram_parameter("xt", [128, nquad], fp16, isOutput=False)
    w0 = nc.declare_dram_parameter("w0", [128, HIDDEN], fp16, isOutput=False)
    w1 = nc.declare_dram_parameter("w1", [HIDDEN, HIDDEN], fp16, isOutput=False)
    w2 = nc.declare_dram_parameter("w2", [HIDDEN, HIDDEN], fp16, isOutput=False)
    w3 = nc.declare_dram_parameter("w3", [HIDDEN, HIDDEN], fp16, isOutput=False)
    w4 = nc.declare_dram_parameter("w4", [HIDDEN, OUTPUT_DIM], fp16, isOutput=False)
    # yt[p, (g//2)*1024 + (g%2)*512 + c*16 + o] = out[sample(p,g,c), o]
    yt = nc.declare_dram_parameter("yt", [128, ncols // 8], fp16, isOutput=True)

    with tile.TileContext(nc) as tc:
        with (
            tc.tile_pool(name="wp", bufs=1) as wp,
            tc.tile_pool(name="io", bufs=1) as io,
            tc.tile_pool(name="hp", bufs=1) as hp,
            tc.tile_pool(name="op", bufs=1) as op,
            tc.tile_pool(name="ps", bufs=1, space="PSUM") as ps,
        ):
            wwarm = wp.tile([128, 128], fp16, tag="wm", name="wwarm")
            nc.gpsimd.memset(wwarm[:, :], 0.0)

            w0s = wp.tile([128, HIDDEN], fp16, tag="w0", name="w0s")
            w1s = wp.tile([HIDDEN, HIDDEN], fp16, tag="w1", name="w1s")
            w2s = wp.tile([HIDDEN, HIDDEN], fp16, tag="w2", name="w2s")
            w3s = wp.tile([HIDDEN, HIDDEN], fp16, tag="w3", name="w3s")
            w4s = wp.tile([HIDDEN, OUTPUT_DIM], fp16, tag="w4", name="w4s")
            wlayer = {1: w1s, 2: w2s, 3: w3s}

            pwarm = ps.tile([128, 1024], fp32, tag="blk", bufs=4, name="pwarm")
            for _ in range(N_WARM):
                nc.tensor.matmul(
                    pwarm[:, 0:128], wwarm[:, :], wwarm[:, 0:128],
                    start=True, stop=True,
                )

            bal = {"act": 0.0, "dve": 0.0}
            EVAC_LOG.clear()

            def evac(dst, src, relu, fd, what=""):
                a, d = ACT_NS(fd), DVE_NS(fd)
                if bal["act"] + a <= bal["dve"] + d:
                    bal["act"] += a
                    EVAC_LOG.append(("A", fd, what))
                    if relu:
                        nc.scalar.activation(dst, src, RELU)
                    else:
                        nc.scalar.copy(out=dst, in_=src)
                else:
                    bal["dve"] += d
                    EVAC_LOG.append(("D", fd, what))
                    if relu:
                        nc.vector.tensor_scalar_max(dst, src, 0.0)
                    else:
                        nc.vector.tensor_copy(dst, src)

            slabs = {}

            def load_slab(g):
                if g >= n_groups:
                    return
                xs = io.tile([128, 1024], fp16, tag="xin", bufs=6, name="xs")
                nc.sync.dma_start(out=xs, in_=xt[:, g * 1024 : (g + 1) * 1024])
                slabs[g] = xs

            # The first matmul needs slab0 cols 0:512 (L0 units m0+m1 read
            # only ql=0 chunks) AND w0. Each DMA's pre-transfer chain is
            # ~SEQ 650 + HWDGE 625 + DGE 650 serialized on HWDGE, so emit
            # exactly those two first (bigger transfer first).
            xs0 = io.tile([128, 1024], fp16, tag="xin", bufs=6, name="xs")
            nc.sync.dma_start(out=xs0[:, 0:512], in_=xt[:, 0:512])
            nc.sync.dma_start(out=w0s, in_=w0[:, :])
            nc.sync.dma_start(out=xs0[:, 512:1024], in_=xt[:, 512:1024])
            slabs[0] = xs0
            nc.sync.dma_start(out=w1s, in_=w1[:, :])
            nc.sync.dma_start(out=w2s, in_=w2[:, :])
            load_slab(1)
            nc.sync.dma_start(out=w3s, in_=w3[:, :])
            nc.sync.dma_start(out=w4s, in_=w4[:, :])

            h = {}  # (layer 1..4, g, m) -> SBUF tile [128, 1024]
            osb = {}
            l4state = {}

            def do_unit(layer, g, m, xs):
                blk = ps.tile([128, 1024], fp32, tag="blk", bufs=4,
                              name=f"p{layer}")
                if layer == 0:
                    for half in range(2):
                        i = 2 * m + half
                        j, ql = i % 4, i // 4
                        nc.tensor.matmul(
                            blk[:, half * 512 : (half + 1) * 512],
                            w0s[32 * j : 32 * j + INPUT_DIM, :],
                            xs[32 * j : 32 * j + INPUT_DIM,
                               ql * 512 : (ql + 1) * 512],
                            start=True, stop=True,
                            tile_position=(32 * j, 0),
                        )
                else:
                    src = h[(layer, g, m)]
                    for half in range(2):
                        nc.tensor.matmul(
                            blk[:, half * 512 : (half + 1) * 512],
                            wlayer[layer][:, :],
                            src[:, half * 512 : (half + 1) * 512],
                            start=True, stop=True,
                        )
                hn = hp.tile([128, 1024], fp16, tag="h", bufs=24,
                             name=f"h{layer + 1}")
                evac(hn[:, :], blk[:, :], relu=True, fd=1024,
                     what=f"h{layer + 1} g{g} m{m}")
                h[(layer + 1, g, m)] = hn

            # L4 (flipped): 32 chunk-matmuls, stationary = h4 chunk, moving =
            # W4^T [128,16]; out [128 samples, 16]. Uses half a rotation tile
            # (the other half unused), keeping the PSUM WAR loop at depth 4.
            # Emission is deferred into the NEXT group's L0 phase so neither
            # evac engine ever queues behind the h4(g,3)->L4-matmul chain.
            def l4_mms(g, chunks=range(32)):
                if g not in l4state:
                    l4state[g] = ps.tile([128, 1024], fp32, tag="blk", bufs=4,
                                         name="l4ps")
                l4ps = l4state[g]
                for c in chunks:
                    m, r = c // 8, c % 8  # h4 tile m, 128-col slice r
                    nc.tensor.matmul(
                        l4ps[:, c * 16 : (c + 1) * 16],
                        h[(4, g, m)][:, r * 128 : (r + 1) * 128],
                        w4s[:, :],
                        start=True, stop=True,
                    )

            def l4_evac(g, lo=0, hi=512):
                l4ps = l4state[g]
                gh = g % 2
                if gh == 0 and lo == 0:
                    osb[g // 2] = op.tile([128, 1024], fp16, tag="osb", bufs=2,
                                          name="osb")
                ob = osb[g // 2]
                evac(ob[:, gh * 512 + lo : gh * 512 + hi], l4ps[:, lo:hi],
                     relu=False, fd=hi - lo, what=f"L4 g{g}")
                q0 = (g // 2) * 1024 + gh * 512
                nc.sync.dma_start(
                    out=yt[:, q0 + lo : q0 + hi], in_=ob[:, gh * 512 + lo : gh * 512 + hi]
                )
                if hi == 512:
                    for m in range(4):
                        del h[(4, g, m)]

            last = n_groups - 1
            for g in range(n_groups):
                load_slab(g + 2)
                xs = slabs.pop(g)
                for layer in range(4):
                    for m in range(4):
                        do_unit(layer, g, m, xs)
                        if g > 0 and (layer, m) == (0, 2):
                            l4_mms(g - 1)
                        if g > 0 and (layer, m) == (1, 0):
                            l4_evac(g - 1)
                        # stage the last group's L4 so the tail only waits
                        # on the final 8 chunks (h4(15,3)) + an FD128 evac
                        if g == last and (layer, m) == (3, 2):
                            l4_mms(g, range(0, 8))
                        if g == last and (layer, m) == (3, 3):
                            l4_mms(g, range(8, 16))
            # tail: finish last group's L4 in small pieces
            l4_evac(last, 0, 256)
            l4_mms(last, range(16, 24))
            l4_evac(last, 256, 384)
            l4_mms(last, range(24, 32))
            l4_evac(last, 384, 512)
    _split_waits(nc)
    return nc


def _split_weights(weights):
    ws = []
    off = 0
    ws.append(weights[off : off + HIDDEN * INPUT_DIM].reshape(HIDDEN, INPUT_DIM))
    off += HIDDEN * INPUT_DIM
    for _ in range(NUM_LAYERS - 1):
        ws.append(weights[off : off + HIDDEN * HIDDEN].reshape(HIDDEN, HIDDEN))
        off += HIDDEN * HIDDEN
    ws.append(weights[off : off + OUTPUT_DIM * HIDDEN].reshape(OUTPUT_DIM, HIDDEN))
    return ws


_NC_CACHE = {}
_PERM_CACHE = {}
EVAC_LOG = []  # (engine, fd, what) in emission order, filled by build()


def make_in_maps(inputs: np.ndarray, weights: np.ndarray):
    ws = _split_weights(np.asarray(weights, dtype=np.float32))
    w0t = np.ascontiguousarray(ws[0].T).astype(np.float16)  # [32, 128]
    wmaps = {
        "w0": np.concatenate([w0t] * 4, axis=0),  # [128, 128], 4 strips
        "w1": np.ascontiguousarray(ws[1].T).astype(np.float16),
        "w2": np.ascontiguousarray(ws[2].T).astype(np.float16),
        "w3": np.ascontiguousarray(ws[3].T).astype(np.float16),
        "w4": np.ascontiguousarray(ws[4].T).astype(np.float16),  # [128, 16]
    }
    in_maps = []
    for i in range(N_CORES):
        xc = inputs[i * B_CORE : (i + 1) * B_CORE]
        xtc = np.ascontiguousarray(xc.T).astype(np.float16)  # [32, B_CORE]
        xq = np.ascontiguousarray(
            xtc.reshape(INPUT_DIM, N_CHUNKS // 4, 4, CHUNK)
            .transpose(2, 0, 1, 3)
            .reshape(128, B_CORE // 4)
        )
        in_maps.append({"xt": xq, **wmaps})
    return in_maps


def _out_perm():
    """sample index for each (p, col) of yt: yt[p, col] = out[perm[p, col], :]
    column o. Returns perm flat index array of shape [128, B_CORE//4 // 16]
    mapping (p, g, gh, c) -> batch sample."""
    if "perm" in _PERM_CACHE:
        return _PERM_CACHE["perm"]
    # yt col = g*1024 + gh*512 + c*16 + o; chunk c (0..31) of group 2*gq+gh
    # maps to h4 tile m = c//8, slice r = c%8; tile m col = r*128 + p,
    # which is packed chunk i = 2m + r//4, offset (r%4)*128 + p;
    # packed chunk i of group g: j = i%4, ql = i//4 -> batch chunk
    # 4*(2g+ql) + j; sample = bchunk*512 + (r%4)*128 + p.
    perm = np.empty((128, N_GROUPS // 2, 2, 32), dtype=np.int64)
    for g in range(N_GROUPS):
        for c in range(32):
            m, r = c // 8, c % 8
            i = 2 * m + r // 4
            j, ql = i % 4, i // 4
            bchunk = 4 * (2 * g + ql) + j
            base = bchunk * 512 + (r % 4) * 128
            perm[:, g // 2, g % 2, c] = base + np.arange(128)
    perm = perm.reshape(128, -1)
    _PERM_CACHE["perm"] = perm
    return perm


def kernel(inputs: np.ndarray, weights: np.ndarray) -> np.ndarray:
    from concourse.bass_utils import run_bass_kernel_spmd

    assert inputs.shape == (B, INPUT_DIM), inputs.shape
    in_maps = make_in_maps(inputs, weights)
    if "nc" not in _NC_CACHE:
        _NC_CACHE["nc"] = build()
    nc = _NC_CACHE["nc"]
    res = run_bass_kernel_spmd(nc, in_maps, list(range(N_CORES)))
    perm = _out_perm()  # [128, ncols/16]
    outs = []
    for ci, r in enumerate(res.results):
        yq = r["yt"]  # [128, B_CORE/4] fp16
        o = np.empty((B_CORE, OUTPUT_DIM), dtype=np.float32)
        cols = yq.reshape(128, -1, 16)  # [p, cblk, o]
        # scatter: out[perm[p, cb], o] = cols[p, cb, o]
        o[perm.reshape(-1)] = cols.reshape(-1, 16).astype(np.float32)
        outs.append(o)
    return np.concatenate(outs, axis=0)
